# revision 36
# baseline (speedup 1.0000x reference)
"""Trainium2 Bass kernel for nn_MultiHeadAttention_70884140253264.

Reference semantics (note: projects q for ALL of q/k/v; k and v inputs are
unused; all 8 heads are identical):
    scale = sqrt(1024)*10
    qp, kp, vp = q@Wq.T, q@Wk.T, q@Wv.T
    att  = softmax(qp @ kp.T / scale) + 1e-5          # [B,S,S]
    x    = tile(att@vp, 8) @ Wo.T + bo                # [B,S,D]
    returns (x, broadcast(att, heads))

Host-side algebraic folds (weights only):
    M1 = Wq.T @ Wk / scale      -> scores = q @ M1 @ q.T
    Wo_sum = sum of 8 head blocks of Wo
    M2 = Wv.T @ Wo_sum.T        -> x = att @ q @ M2 + bo

Sharding: 512 query rows per core (cores 0-3 batch 0, cores 4-7 batch 1).
Each core computes, in fp16 on the PE:
    tT    [D, 512]  = M1.T-tiles @ qT_shard        (lhsT=M1, rhs=qT cols)
  then per 128-row query m-tile (fully pipelined):
    scores[128, S]  = tT-tiles  @ qT_full
    att   = exp(scores) * (1/rowsum) + 1e-5        (no max-sub: |scores|<~0.7)
    attbT [S, 128]  = DMA-xbar transpose of att (fp16)
    y1T   [D, 128]  = q-tiles   @ attbT            (= (att @ q).T)
    x     [128, D]  = y1T-tiles @ M2 + bo
att (fp16) and x (fp16) are DMA'd out (upcast on host); the 8 identical heads of att_head are a
host-side broadcast view.
"""

import numpy as np

import concourse.bacc as bacc
import concourse.tile as tile
import concourse.mybir as mybir
from concourse.bass_utils import run_bass_kernel_spmd

B, S, DIM = 2, 2048, 1024
N_HEADS = 8
N_CORES = 8
BLOCKS_PER_BATCH = N_CORES // B
BLK = S // BLOCKS_PER_BATCH  # 512 query rows per core
P = 128
MT = BLK // P    # 4  m-tiles of query rows per core
NT = S // 512    # 4  512-wide key chunks
KT_D = DIM // P  # 8  k-tiles over feature dim
KT_S = S // P    # 16 k-tiles over key dim
FP32 = mybir.dt.float32
F16 = mybir.dt.float16
NP_F16 = np.float16
INV_SCALE = 1.0 / (np.sqrt(np.float64(DIM)) * 10.0)

_CACHED_NC = None


def _build_nc():
    nc = bacc.Bacc("TRN2", target_bir_lowering=False, debug=False,
                   num_devices=N_CORES)

    qTs_d = nc.dram_tensor("qTs", [DIM, BLK], F16, kind="ExternalInput")
    qT_d = nc.dram_tensor("qT", [DIM, S], F16, kind="ExternalInput")
    qn_d = nc.dram_tensor("qn", [S, DIM], F16, kind="ExternalInput")
    m1_d = nc.dram_tensor("m1", [DIM, DIM], F16, kind="ExternalInput")
    m2_d = nc.dram_tensor("m2", [DIM, DIM], F16, kind="ExternalInput")
    bo_d = nc.dram_tensor("bo", [P, DIM], FP32, kind="ExternalInput")
    att_d = nc.dram_tensor("att_out", [BLK, S], F16, kind="ExternalOutput")
    x_d = nc.dram_tensor("x_out", [BLK, DIM], F16, kind="ExternalOutput")

    with tile.TileContext(nc) as tc:
        with (
            tc.tile_pool(name="persist", bufs=1) as persist,
            tc.tile_pool(name="attf", bufs=2) as attf_pool,
            tc.tile_pool(name="attb", bufs=2) as attb_pool,
            tc.tile_pool(name="small", bufs=4) as small,
            tc.tile_pool(name="xout", bufs=2) as x_pool,
            tc.tile_pool(name="abt", bufs=2) as abt_pool,
            tc.tile_pool(name="mm", bufs=5, space="PSUM") as mm_pool,
            tc.tile_pool(name="acc", bufs=3, space="PSUM") as acc_pool,
        ):
            # loads in dependency-priority order: m1+qTs gate the first
            # matmuls; qT gates scores; qn gates y1T; m2/bo gate only x.
            m1_sb = persist.tile([P, KT_D, DIM], F16)
            qTs_sb = persist.tile([P, KT_D, BLK], F16)
            qT_sb = persist.tile([P, KT_D, S], F16)
            m2_sb = persist.tile([P, KT_D, DIM], F16)
            qn_sb = persist.tile([P, KT_S, DIM], F16)
            def load_chunks(sb, dram, chunks):
                # batched kt-block loads: [128, n_kt, F] from n_kt row-blocks
                kt0 = 0
                for n in chunks:
                    src = dram[kt0 * P:(kt0 + n) * P, :]
                    if n > 1:
                        src = src.rearrange("(kt p) e -> p kt e", p=P)
                        nc.sync.dma_start(out=sb[:, kt0:kt0 + n, :], in_=src)
                    else:
                        nc.sync.dma_start(out=sb[:, kt0, :], in_=src)
                    kt0 += n

            def load_chunk(sb, dram, kt0, n):
                src = dram[kt0 * P:(kt0 + n) * P, :]
                if n > 1:
                    src = src.rearrange("(kt p) e -> p kt e", p=P)
                    nc.sync.dma_start(out=sb[:, kt0:kt0 + n, :], in_=src)
                else:
                    nc.sync.dma_start(out=sb[:, kt0, :], in_=src)

            # small first chunks so the PE's first matmuls start early; the
            # rest batched to amortize the ~0.6us HWDGE trigger cost. m1/qTs
            # interleaved since tT consumes them pairwise by kt. The very
            # first lhsT block [128,128] loads alone so Ldweights can fire
            # as soon as possible.
            nc.sync.dma_start(out=m1_sb[:, 0, 0:P], in_=m1_d[0:P, 0:P])
            load_chunk(qTs_sb, qTs_d, 0, 1)
            nc.sync.dma_start(out=m1_sb[:, 0, P:DIM], in_=m1_d[0:P, P:DIM])
            for kt0, n in [(1, 1), (2, 2), (4, 2), (6, 2)]:
                load_chunk(m1_sb, m1_d, kt0, n)
                load_chunk(qTs_sb, qTs_d, kt0, n)
            # qT loaded by 512-column chunks: scores' nt-th PSUM group only
            # needs column chunk nt, so the PE can start scoring while the
            # rest of qT is still in flight.
            for ntc in range(NT):
                csl = slice(ntc * 512, (ntc + 1) * 512)
                nc.sync.dma_start(
                    out=qT_sb[:, :, csl],
                    in_=qT_d[:, csl].rearrange("(kt p) s -> p kt s", p=P))
            load_chunks(qn_sb, qn_d, [4, 4, 4, 4])
            load_chunks(m2_sb, m2_d, [4, 4])
            bo_sb = persist.tile([P, DIM], FP32)
            nc.sync.dma_start(out=bo_sb, in_=bo_d[:, :])

            # tT[e, s] = (q_shard @ M1).T : lhsT = M1 k-tiles, rhs = qTs.
            # kt-outermost over 4 concurrent PSUM groups so the PE paces with
            # the per-kt DMA arrivals of m1/qTs instead of waiting for all.
            tT_sb = persist.tile([P, KT_D, BLK], F16)
            for half in range(2):
                pss = [mm_pool.tile([P, 512], FP32, tag="mm", name=f"ps_tt{half}_{j}")
                       for j in range(4)]
                for kt in range(KT_D):
                    for j in range(4):
                        et = half * 4 + j
                        nc.tensor.matmul(
                            pss[j],
                            m1_sb[:, kt, et * P:(et + 1) * P],
                            qTs_sb[:, kt, :],
                            start=(kt == 0), stop=(kt == KT_D - 1),
                        )
                for j in range(4):
                    nc.scalar.copy(out=tT_sb[:, half * 4 + j, :], in_=pss[j])

            y1T_sb = persist.tile([P, KT_D, BLK], F16)
            attbT_tiles = [None] * MT
            att_f_tiles = [None] * MT
            zp_tiles = [None] * MT

            def scores_stage(mt, kt_outer):
                msl = slice(mt * P, (mt + 1) * P)
                att_f = attf_pool.tile([P, S], FP32, tag="attf")
                zp = small.tile([P, NT], FP32, tag="zp")
                att_f_tiles[mt] = att_f
                zp_tiles[mt] = zp
                if kt_outer:
                    # pace with qT per-kt DMA arrivals (4 concurrent groups)
                    pss = [mm_pool.tile([P, 512], FP32, tag="mm",
                                        name=f"ps_sc{mt}_{nt}")
                           for nt in range(NT)]
                    for kt in range(KT_D):
                        for nt in range(NT):
                            nc.tensor.matmul(
                                pss[nt],
                                tT_sb[:, kt, msl],
                                qT_sb[:, kt, nt * 512:(nt + 1) * 512],
                                start=(kt == 0), stop=(kt == KT_D - 1),
                            )
                    for nt in range(NT):
                        nc.scalar.activation(
                            out=att_f[:, nt * 512:(nt + 1) * 512],
                            in_=pss[nt],
                            func=mybir.ActivationFunctionType.Exp,
                            scale=float(INV_SCALE),
                            accum_out=zp[:, nt:nt + 1],
                        )
                else:
                    for nt in range(NT):
                        ps = mm_pool.tile([P, 512], FP32, tag="mm")
                        for kt in range(KT_D):
                            nc.tensor.matmul(
                                ps,
                                tT_sb[:, kt, msl],
                                qT_sb[:, kt, nt * 512:(nt + 1) * 512],
                                start=(kt == 0), stop=(kt == KT_D - 1),
                            )
                        nc.scalar.activation(
                            out=att_f[:, nt * 512:(nt + 1) * 512],
                            in_=ps,
                            func=mybir.ActivationFunctionType.Exp,
                            scale=float(INV_SCALE),
                            accum_out=zp[:, nt:nt + 1],
                        )

            def softmax_transpose_stage(mt):
                msl = slice(mt * P, (mt + 1) * P)
                att_f = att_f_tiles[mt]
                z = small.tile([P, 1], FP32, tag="z")
                nc.vector.reduce_sum(out=z, in_=zp_tiles[mt],
                                     axis=mybir.AxisListType.X)
                rz = small.tile([P, 1], FP32, tag="rz")
                nc.vector.reciprocal(out=rz, in_=z)

                # single fp16 normalization feeds the PE (via xbar transpose)
                # and the att output DMA
                ab = attb_pool.tile([P, S], F16, tag="attb")
                nc.vector.tensor_scalar(
                    out=ab, in0=att_f,
                    scalar1=rz, scalar2=1e-5,
                    op0=mybir.AluOpType.mult, op1=mybir.AluOpType.add,
                )
                # transpose att[mt] via the DMA xbar: [128, S] fp16 ->
                # contiguous [128, KT_S, 128] (keys on partitions)
                abt = abt_pool.tile([P, KT_S, P], F16, tag="abt")
                nc.sync.dma_start_transpose(abt[:, :, :], ab[:, :])
                attbT_tiles[mt] = abt
                nc.sync.dma_start(out=att_d[msl, :], in_=ab)

            def y1t_stage(mt):
                msl = slice(mt * P, (mt + 1) * P)
                for half in range(2):
                    ps = acc_pool.tile([P, 4 * P], FP32, tag="acc")
                    for j in range(4):
                        dt = half * 4 + j
                        for kt in range(KT_S):
                            nc.tensor.matmul(
                                ps[:, j * P:(j + 1) * P],
                                qn_sb[:, kt, dt * P:(dt + 1) * P],
                                attbT_tiles[mt][:, kt, :],
                                start=(kt == 0), stop=(kt == KT_S - 1),
                            )
                    nc.scalar.copy(
                        out=y1T_sb[:, half * 4:(half + 1) * 4, msl],
                        in_=ps.rearrange("p (k c) -> p k c", k=4),
                    )

            def x_stage(mt, chunks=2):
                # the final m-tile uses finer chunks so the tail
                # (last matmul -> add -> DMA -> drain) is as short as possible
                msl = slice(mt * P, (mt + 1) * P)
                xt = x_pool.tile([P, DIM], F16, tag="xt")
                cw = DIM // chunks
                for nt in range(chunks):
                    ns = slice(nt * cw, (nt + 1) * cw)
                    ps = acc_pool.tile([P, 512], FP32, tag="acc")
                    for kt in range(KT_D):
                        nc.tensor.matmul(
                            ps[:, 0:cw],
                            y1T_sb[:, kt, msl],
                            m2_sb[:, kt, ns],
                            start=(kt == 0), stop=(kt == KT_D - 1),
                        )
                    nc.vector.tensor_add(out=xt[:, ns], in0=ps[:, 0:cw],
                                         in1=bo_sb[:, ns])
                    nc.sync.dma_start(out=x_d[msl, ns], in_=xt[:, ns])

            # software-pipelined PE order: scores runs one m-tile ahead of
            # its softmax/transpose consumers; x trails by one more stage so
            # the PE never waits on the ACT/DVE chains.
            scores_stage(0, kt_outer=False)
            scores_stage(1, kt_outer=False)
            softmax_transpose_stage(0)
            y1t_stage(0)
            scores_stage(2, kt_outer=False)
            x_stage(0)
            softmax_transpose_stage(1)
            y1t_stage(1)
            scores_stage(3, kt_outer=False)
            x_stage(1)
            softmax_transpose_stage(2)
            y1t_stage(2)
            softmax_transpose_stage(3)
            x_stage(2)
            y1t_stage(3)
            x_stage(3, chunks=4)

    nc.compile()
    return nc


def get_nc():
    global _CACHED_NC
    if _CACHED_NC is None:
        _CACHED_NC = _build_nc()
    return _CACHED_NC


def make_in_maps(q, Wq, Wk, Wv, Wo, bo):
    m1 = Wq.T.astype(np.float64) @ Wk.astype(np.float64)
    wo_sum = Wo.reshape(DIM, N_HEADS, DIM).sum(axis=1, dtype=np.float64)
    m2 = Wv.T.astype(np.float64) @ wo_sum.T
    m1 = m1.astype(NP_F16)
    m2 = m2.astype(NP_F16)
    bo_bc = np.ascontiguousarray(
        np.broadcast_to(bo.astype(np.float32), (P, DIM)))

    in_maps = []
    for c in range(N_CORES):
        b = c // BLOCKS_PER_BATCH
        qb = c % BLOCKS_PER_BATCH
        qb_sl = slice(qb * BLK, (qb + 1) * BLK)
        q_b = q[b].astype(NP_F16)
        qT_b = np.ascontiguousarray(q_b.T)
        in_maps.append({
            "qTs": np.ascontiguousarray(qT_b[:, qb_sl]),
            "qT": qT_b,
            "qn": q_b,
            "m1": m1,
            "m2": m2,
            "bo": bo_bc,
        })
    return in_maps


def kernel(q, k, v, Wq, Wk, Wv, Wo, bo):
    q = np.asarray(q, dtype=np.float32)
    nc = get_nc()
    in_maps = make_in_maps(q, np.asarray(Wq), np.asarray(Wk), np.asarray(Wv),
                           np.asarray(Wo), np.asarray(bo))
    res = run_bass_kernel_spmd(nc, in_maps, core_ids=list(range(N_CORES)))

    att = np.empty((B, S, S), np.float32)
    x = np.empty((B, S, DIM), np.float32)
    for c in range(N_CORES):
        b = c // BLOCKS_PER_BATCH
        qb = c % BLOCKS_PER_BATCH
        sl = slice(qb * BLK, (qb + 1) * BLK)
        att[b, sl] = res.results[c]["att_out"].astype(np.float32)
        x[b, sl] = res.results[c]["x_out"].astype(np.float32)

    att_head = np.broadcast_to(att[:, None], (B, N_HEADS, S, S))
    return (x, att_head)
